# revision 5
# baseline (speedup 1.0000x reference)
# Causal self-attention (B=8, T=1024, C=1024, H=16, D=64) on 8 trn2 NeuronCores.
# Sharding: data-parallel over batch — core i computes batch element i entirely
# (weights replicated, no collectives).
#
# Per-core pipeline:
#   0. x cast-loaded bf16 in 4 chunks, PE-transposed to xT[c,t] pipelined
#      behind the DMA. Each transpose is evacuated twice: bf16 (v-proj lhsT)
#      and fp8e4m3 scaled x16 (qk-proj operand). Dummy matmuls keep the PE
#      HAM clock gate at 8/8; a tiny exp preloads the ACT spline table early.
#   1. v proj (bf16) per (ti, jvt): lhsT=xT chunk, rhs=W_v slice -> vp (ones
#      col at D per head so attn@v also yields the softmax denominator l).
#      v-bias folds into the output bias (b~ = b_v @ W_out + b_out, on PE).
#   2. qk proj in fp8 DoubleRow (2 contraction rows per cell, ~1.4x): weights
#      staged bf16 then scale-cast x512 to fp8 on DVE; q-bias (x8192) added
#      by a K=1 bf16 matmul inside the accumulation; k-bias dropped (softmax
#      shift-invariance). Scores come out scaled by 8192^2, absorbed into the
#      exp scale. Accuracy: simulated rel err 0.008 vs 0.02 budget.
#   3. attention per head-pair hp, 512-query tile it: scores sT[j,i] (K=64,
#      two heads at partition bases 0/64 run concurrently), exp on ACT ->
#      pT bf16, diag-block causal fix = bf16 multiply by a 0/1 mask, attn@v
#      with lhsT padded to M=128 (keeps FWL on; row 64 = l, rows 65+ junk),
#      l -> reciprocal -> oT. Remaining qk/v-proj/bias-fold units are pulled
#      from a paced queue between attention steps so the PE never idles.
#   4. out proj (bf16) per ti: lhsT=oT chunk, rhs=W_out, bias via K=1 matmul.

import numpy as np
import ml_dtypes
from contextlib import ExitStack

import concourse.bass as bass
import concourse.bacc as bacc
import concourse.mybir as mybir
import concourse.tile as tile
from concourse import bass_utils

FP32 = mybir.dt.float32
BF16 = mybir.dt.bfloat16
FP8 = mybir.dt.float8e4

B, T, C = 8, 1024, 1024
H, D = 16, 64
N_CORES = 8
CCH = C // 128
TCH = T // 128
VP_W = H * (D + 1) + 63  # head stride 65, +63 pad so attn@v lhsT can be M=128
XS = 16.0                # fp8 scale for x
WS = 512.0               # fp8 scale for W_q/W_k
EXP_SCALE = 0.125 / (XS * XS * WS * WS)


def build_nc():
    nc = bacc.Bacc("TRN2", debug=False, num_devices=N_CORES)

    x_d = nc.dram_tensor("x_b", [T, C], FP32, kind="ExternalInput").ap()
    wq_d = nc.dram_tensor("w_qkv", [C, 3 * C], FP32, kind="ExternalInput").ap()
    bq_d = nc.dram_tensor("b_qkv", [1, 3 * C], FP32, kind="ExternalInput").ap()
    wo_d = nc.dram_tensor("w_out", [C, C], FP32, kind="ExternalInput").ap()
    bo_d = nc.dram_tensor("b_out", [1, C], FP32, kind="ExternalInput").ap()
    id_d = nc.dram_tensor("ident", [128, 128], BF16, kind="ExternalInput").ap()
    mk_d = nc.dram_tensor("mask01", [128, 128], BF16, kind="ExternalInput").ap()
    out_d = nc.dram_tensor("out_b", [T, C], FP32, kind="ExternalOutput").ap()

    with tile.TileContext(nc) as tc, ExitStack() as ctx:
        consts = ctx.enter_context(tc.tile_pool(name="consts", bufs=1))
        wpool = ctx.enter_context(tc.tile_pool(name="weights", bufs=1))
        apool = ctx.enter_context(tc.tile_pool(name="acts", bufs=1))
        xpool = ctx.enter_context(tc.tile_pool(name="xstage", bufs=2))
        wstage = ctx.enter_context(tc.tile_pool(name="wstage", bufs=2))
        ppool = ctx.enter_context(tc.tile_pool(name="ppool", bufs=4))
        lpool = ctx.enter_context(tc.tile_pool(name="lpool", bufs=2))
        rbpool = ctx.enter_context(tc.tile_pool(name="rbpool", bufs=2))
        outs = ctx.enter_context(tc.tile_pool(name="outs", bufs=2))
        # PSUM, 8 banks: "ps" [128,2,512]x2 = 4 (scores / plb / openers /
        # out-proj), "po" [128,512]x3 = 3 (attn out + transposes), "aux" x1.
        PS = ctx.enter_context(tc.tile_pool(name="PS", bufs=2, space="PSUM"))

        # ---- host constants on the HWDGE queue (no Q7 time) ----
        ident = consts.tile([128, 128], BF16, tag="ident", name="ident")
        nc.sync.dma_start(out=ident, in_=id_d)
        mask01 = consts.tile([128, 128], BF16, tag="mask01", name="mask01")
        nc.sync.dma_start(out=mask01, in_=mk_d)

        # ---- vector-engine constants ----
        ones_row = consts.tile([1, 512], BF16, tag="ones_row", name="ones_row")
        nc.vector.memset(ones_row, 1.0)
        ones64 = consts.tile([1, 64], BF16, tag="ones64", name="ones64")
        nc.vector.memset(ones64, 1.0)
        scratch = consts.tile([128, 512], BF16, tag="scratch", name="scratch")
        nc.vector.memset(scratch, 1.0)
        warm_exp = consts.tile([1, 16], BF16, tag="warm_exp", name="warm_exp")
        nc.scalar.activation(out=warm_exp, in_=scratch[0:1, 0:16],
                             func=mybir.ActivationFunctionType.Exp, scale=0.125)

        # ---- persistent activations ----
        xTall = apool.tile([128, CCH, T], BF16, tag="xTall", name="xTall")
        xT8 = apool.tile([128, CCH, T], FP8, tag="xT8", name="xT8")
        qkT = [apool.tile([128, T], BF16, tag=f"qkT{jt}", name=f"qkT{jt}")
               for jt in range(16)]
        vp = [apool.tile([128, VP_W], BF16, tag=f"vp{t_}", name=f"vp{t_}")
              for t_ in range(TCH)]
        oT = [apool.tile([128, T], BF16, tag=f"oT{hc}", name=f"oT{hc}")
              for hc in range(CCH)]
        btilde = consts.tile([1, C], BF16, tag="btilde", name="btilde")

        # ---- gpsimd cast-DMAs in consumption order ----
        bqkv_sb = consts.tile([1, 3 * C], BF16, tag="bqkv", name="bqkv_sb")
        nc.gpsimd.dma_start(out=bqkv_sb, in_=bq_d)
        # q-bias feeds fp8-scaled scores: pre-scale by XS*WS
        nc.vector.tensor_scalar(
            out=bqkv_sb[0:1, 0:C], in0=bqkv_sb[0:1, 0:C],
            scalar1=XS * WS, scalar2=None, op0=mybir.AluOpType.mult)
        bout_sb = consts.tile([1, C], BF16, tag="bout", name="bout_sb")
        nc.gpsimd.dma_start(out=bout_sb, in_=bo_d)

        xs = []

        def load_x_block(xb):
            t_ = xpool.tile([128, 2, C], BF16, tag="xs", name=f"xs{xb}")
            nc.gpsimd.dma_start(
                out=t_,
                in_=x_d[xb * 256:(xb + 1) * 256, :].rearrange(
                    "(k p) c -> p k c", p=128))
            xs.append(t_)

        wv_col = []
        for jvt in range(2):
            t_ = wpool.tile([128, CCH, 512], BF16, tag=f"wv{jvt}", name=f"wv{jvt}")
            wv_col.append(t_)

        def load_wv(jvt):
            src = wq_d[:, 2 * C + jvt * 512: 2 * C + (jvt + 1) * 512]
            nc.gpsimd.dma_start(
                out=wv_col[jvt], in_=src.rearrange("(cc p) j -> p cc j", p=128))

        # q/k blocks: DMA bf16 into a staging ring, scale-cast x512 to fp8
        wqk8 = {}

        def load_wqk_block(kind, b):
            stg = wstage.tile([128, CCH, 256], BF16, tag="wstg", name=f"stg{kind}{b}")
            base = b * 256 if kind == "q" else C + b * 256
            nc.gpsimd.dma_start(
                out=stg,
                in_=wq_d[:, base:base + 256].rearrange("(cc p) j -> p cc j", p=128))
            t8 = wpool.tile([128, CCH, 256], FP8, tag=f"w8{kind}{b}",
                            name=f"w8{kind}{b}")
            nc.vector.tensor_scalar(out=t8, in0=stg, scalar1=WS, scalar2=None,
                                    op0=mybir.AluOpType.mult)
            wqk8[(kind, b)] = t8

        load_x_block(0)
        load_x_block(1)
        load_wv(0)
        load_wqk_block("q", 0)
        load_wqk_block("k", 0)
        load_x_block(2)
        load_x_block(3)
        load_wqk_block("q", 1)
        load_wqk_block("k", 1)
        load_wv(1)
        load_wqk_block("q", 2)
        load_wqk_block("k", 2)
        load_wqk_block("q", 3)
        load_wqk_block("k", 3)
        bv_pc = consts.tile([128, CCH], BF16, tag="bv_pc", name="bv_pc")
        nc.gpsimd.dma_start(
            out=bv_pc, in_=bq_d[:, 2 * C:3 * C].rearrange("x (cc p) -> p (x cc)", p=128))
        wo_col = wpool.tile([128, CCH, C], BF16, tag="wo", name="wo")
        nc.gpsimd.dma_start(
            out=wo_col, in_=wo_d.rearrange("(cc p) j -> p cc j", p=128))

        n_dummy = [0]

        def dummy_mm():
            # PE heartbeat: keeps the HAM activity window non-idle so the
            # clock gate stays at 8/8. Result is never read.
            ps = PS.tile([128, 512], FP32, tag="aux", name=f"dmy{n_dummy[0]}",
                         bufs=1)
            n_dummy[0] += 1
            nc.tensor.matmul(out=ps, lhsT=scratch[:, 0:128], rhs=scratch,
                             start=True, stop=True)

        # vp ones columns (denominator trick) + zero pad tail
        for ti in range(TCH):
            vcol = vp[ti][:, 0:H * (D + 1)].rearrange("p (h d) -> p h d", h=H)
            nc.vector.memset(vcol[:, :, D:D + 1], 1.0)
            nc.vector.memset(vp[ti][:, H * (D + 1):], 0.0)

        # ---- unit emitters ----
        def transpose_unit(ti, heartbeat=True):
            # 8 PE transposes of token chunk ti; dual evacuation bf16 + fp8
            for cc in range(CCH):
                pt = PS.tile([128, 128], BF16, tag="po", name="tp", bufs=3)
                nc.tensor.transpose(
                    out=pt, in_=xs[ti // 2][:, ti % 2, cc * 128:(cc + 1) * 128],
                    identity=ident)
                nc.vector.tensor_copy(
                    out=xTall[:, cc, ti * 128:(ti + 1) * 128], in_=pt)
                nc.scalar.activation(
                    out=xT8[:, cc, ti * 128:(ti + 1) * 128], in_=pt,
                    func=mybir.ActivationFunctionType.Copy, scale=XS)
                if heartbeat and cc in (3, 7):
                    dummy_mm()

        def vproj_unit(ti, jvt, tag):
            ps = PS.tile([128, 512], FP32, tag=tag, name=f"psv{ti}_{jvt}",
                         bufs=1 if tag == "aux" else None)
            for cc in range(CCH):
                nc.tensor.matmul(
                    out=ps,
                    lhsT=xTall[:, cc, ti * 128:(ti + 1) * 128],
                    rhs=wv_col[jvt][:, cc, :],
                    start=(cc == 0), stop=(cc == CCH - 1))
            vcol = vp[ti][:, 0:H * (D + 1)].rearrange("p (h d) -> p h d", h=H)
            nc.vector.tensor_copy(
                out=vcol[:, jvt * 8:(jvt + 1) * 8, 0:D],
                in_=ps.rearrange("p (h d) -> p h d", h=8))

        def qk_group(jt, half, tag):
            # fp8 DoubleRow: 4 virtual chunks of K=256 (2 c-blocks per cell)
            sl = slice(half * 512, (half + 1) * 512)
            ps = PS.tile([128, 512], FP32, tag=tag, name=f"psqk{jt}_{half}",
                         bufs=1 if tag == "aux" else None)
            kind = "q" if jt < 8 else "k"
            jq = jt if jt < 8 else jt - 8
            blk = wqk8[(kind, jq // 2)]
            u = jq % 2
            has_bias = jt < 8
            for v in range(4):
                nc.tensor.matmul(
                    out=ps,
                    lhsT=blk[:, 2 * v:2 * v + 2, u * 128:(u + 1) * 128],
                    rhs=xT8[:, 2 * v:2 * v + 2, sl],
                    start=(v == 0), stop=(v == 3 and not has_bias),
                    perf_mode=mybir.MatmulPerfMode.DoubleRow)
            if has_bias:
                nc.tensor.matmul(
                    out=ps, lhsT=bqkv_sb[0:1, jt * 128:(jt + 1) * 128],
                    rhs=ones_row, start=False, stop=True)
            nc.vector.tensor_copy(out=qkT[jt][:, sl], in_=ps)

        def btilde_unit(half, tag):
            sl = slice(half * 512, (half + 1) * 512)
            ps = PS.tile([128, 512], FP32, tag=tag, name=f"psbt{half}",
                         bufs=1 if tag == "aux" else None)
            for cc in range(CCH):
                nc.tensor.matmul(
                    out=ps[0:1, :], lhsT=bv_pc[:, cc:cc + 1],
                    rhs=wo_col[:, cc, sl],
                    start=(cc == 0), stop=(cc == CCH - 1))
            nc.vector.tensor_tensor(
                out=btilde[0:1, sl], in0=ps[0:1, :], in1=bout_sb[0:1, sl],
                op=mybir.AluOpType.add)

        # ---- opening ----
        for _ in range(4):
            dummy_mm()
        for ti in range(4):
            transpose_unit(ti)
        for ti in range(4):
            vproj_unit(ti, 0, "ps")
            if ti % 2 == 1:
                dummy_mm()
        qk_group(0, 0, "ps")
        qk_group(8, 0, "ps")

        # ---- filler queue (paced through the attention phase) ----
        filler = [("tp", ti, None) for ti in range(4, 8)]
        filler += [("vp", ti, 0) for ti in range(4, 8)]
        filler += [("qk", 0, 1), ("qk", 8, 1)]
        for p in range(1, 8):
            for half in range(2):
                filler.append(("qk", p, half))
                filler.append(("qk", 8 + p, half))
            if p == 3:
                filler += [("vp", ti, 1) for ti in range(TCH)]
            if p == 5:
                filler += [("bt", 0, None), ("bt", 1, None)]
        fill_pos = [0]
        emitted = set()

        def emit_filler(tag):
            if fill_pos[0] >= len(filler):
                dummy_mm()
                return
            kind, a, b = filler[fill_pos[0]]
            fill_pos[0] += 1
            emitted.add((kind, a, b))
            if kind == "qk":
                qk_group(a, b, tag)
            elif kind == "vp":
                vproj_unit(a, b, tag)
            elif kind == "tp":
                transpose_unit(a, heartbeat=False)
            else:
                btilde_unit(a, tag)

        def require(*units):
            while any(u not in emitted for u in units):
                if fill_pos[0] >= len(filler):
                    break
                emit_filler("aux")

        total_jc = 96
        g_jc = [0]

        # ---- attention ----
        for hp in range(8):
            h0, h1 = 2 * hp, 2 * hp + 1
            qk_q, qk_k = qkT[hp], qkT[8 + hp]
            for it in range(2):
                if (hp, it) != (0, 0):
                    require(("qk", hp, it), ("qk", 8 + hp, it))
                if it == 1:
                    require(*[("vp", ti, 0) for ti in range(4, 8)])
                if hp >= 4:
                    require(*[("vp", ti, 1) for ti in range(TCH)])
                njc = 4 * (it + 1)
                po2 = [PS.tile([128, 512], FP32, tag="po", name=f"po{hx}", bufs=3)
                       for hx in range(2)]
                for jc in range(njc):
                    s0 = max(0, jc * 128 - it * 512)
                    ps = PS.tile([128, 2, 512], FP32, tag="ps", name="pss")
                    for hx in range(2):
                        prow = slice(hx * 64, hx * 64 + 64)
                        nc.tensor.matmul(
                            out=ps[:, hx, s0:512],
                            lhsT=qk_k[prow, jc * 128:(jc + 1) * 128],
                            rhs=qk_q[prow, it * 512 + s0:(it + 1) * 512],
                            start=True, stop=True)
                    pT = ppool.tile([128, 2, 512], BF16, tag="pT", name="pT")
                    nc.scalar.activation(
                        out=pT[:, :, s0:512], in_=ps[:, :, s0:512],
                        func=mybir.ActivationFunctionType.Exp, scale=EXP_SCALE)
                    if jc >= it * 4:  # diagonal block: zero the upper triangle
                        nc.vector.tensor_tensor(
                            out=pT[:, :, s0:s0 + 128],
                            in0=pT[:, :, s0:s0 + 128],
                            in1=mask01[:, None, :].to_broadcast([128, 2, 128]),
                            op=mybir.AluOpType.mult)
                    g_jc[0] += 1
                    while fill_pos[0] * total_jc < len(filler) * g_jc[0] \
                            and fill_pos[0] < len(filler):
                        emit_filler("aux")
                    for hx, h in enumerate((h0, h1)):
                        nc.tensor.matmul(
                            out=po2[hx][:, s0:512],
                            lhsT=vp[jc][:, h * (D + 1):h * (D + 1) + 128],
                            rhs=pT[:, hx, s0:512],
                            start=(jc == 0), stop=(jc == njc - 1),
                            skip_group_check=True)
                # normalize: row 64 of po = l = sum_j p
                for hx in range(2):
                    po = po2[hx]
                    l_sb = lpool.tile([1, 512], BF16, tag="l", name="l")
                    nc.scalar.copy(out=l_sb, in_=po[64:65, :])
                    plb = PS.tile([64, 512], FP32, tag="ps", name="plb")
                    nc.tensor.matmul(out=plb, lhsT=ones64, rhs=l_sb,
                                     start=True, stop=True)
                    rb = rbpool.tile([64, 512], FP32, tag="rb", name="rb")
                    nc.vector.reciprocal_approx_fast(out=rb, in_=plb)
                    prow = slice(hx * 64, hx * 64 + 64)
                    nc.vector.tensor_tensor(
                        out=oT[hp][prow, it * 512:(it + 1) * 512],
                        in0=po[0:64, :], in1=rb, op=mybir.AluOpType.mult)

        while fill_pos[0] < len(filler):
            emit_filler("aux")

        # ---- output projection ----
        for ti in range(TCH):
            ot = outs.tile([128, C], FP32, tag="ot", name="ot")
            for half in range(2):
                sl = slice(half * 512, (half + 1) * 512)
                ps = PS.tile([128, 512], FP32, tag="ps", name="pso")
                for hc in range(CCH):
                    nc.tensor.matmul(
                        out=ps,
                        lhsT=oT[hc][:, ti * 128:(ti + 1) * 128],
                        rhs=wo_col[:, hc, sl],
                        start=(hc == 0), stop=False)
                nc.tensor.matmul(
                    out=ps, lhsT=ones_row[0:1, 0:128], rhs=btilde[0:1, sl],
                    start=False, stop=True)
                nc.vector.tensor_copy(out=ot[:, sl], in_=ps)
            nc.sync.dma_start(out=out_d[ti * 128:(ti + 1) * 128, :], in_=ot)

    nc.compile()
    nc.finalize()
    return nc


_CACHE = {}


def _host_consts():
    ident = np.eye(128, dtype=ml_dtypes.bfloat16)
    ii = np.arange(128)
    mask01 = (ii[None, :] >= ii[:, None]).astype(ml_dtypes.bfloat16)
    return ident, mask01


def kernel(x, W_qkv, b_qkv, W_out, b_out):
    if "nc" not in _CACHE:
        _CACHE["nc"] = build_nc()
    nc = _CACHE["nc"]
    x = np.ascontiguousarray(np.asarray(x, dtype=np.float32))
    ident, mask01 = _host_consts()
    in_maps = [
        {
            "x_b": x[i],
            "w_qkv": np.ascontiguousarray(np.asarray(W_qkv, np.float32)),
            "b_qkv": np.ascontiguousarray(np.asarray(b_qkv, np.float32).reshape(1, -1)),
            "w_out": np.ascontiguousarray(np.asarray(W_out, np.float32)),
            "b_out": np.ascontiguousarray(np.asarray(b_out, np.float32).reshape(1, -1)),
            "ident": ident,
            "mask01": mask01,
        }
        for i in range(N_CORES)
    ]
    res = bass_utils.run_bass_kernel_spmd(nc, in_maps, core_ids=list(range(N_CORES)))
    return np.stack([r["out_b"] for r in res.results]).astype(np.float32)
